# revision 50
# baseline (speedup 1.0000x reference)
"""Trainium2 Bass kernel for ANHP multi-head self-attention.

Problem: out[b] = softmax(exp((x Wq + bq)(x Wk + bk)^T / sqrt(dh)) + causal_soft_mask) (x Wv + bv)
Shapes: B=8, S=1024, FEAT=HID=1024, H=8 heads, DH=128.

Sharding: pure data parallel - batch element b -> NeuronCore b. No collectives.

Per-core dataflow (v10):
  - host passes xT = x.T packed partition-major; first-needed DMA pieces are
    fine-grained (128KB) and spread across the sync/gpsimd/scalar queues in
    demand order (each hwdge queue only keeps ~2 descriptors in flight);
    later-needed weights are emitted from the scalar queue between head
    fronts so their transfers don't steal early HBM bandwidth,
  - qT/kT are produced as [dh, S] per head (projection with W as lhsT);
    scores are computed directly transposed: scT[sk, sq] = k_sk.q_sq/sqrt(dh)
    into 2-bank PSUM tiles so exp1 is one activation per strip,
  - st packed trapezoid [128, 4608] f32; the causal -128 additive mask is
    applied to the diagonal st block between exp1 and exp2 (DVE, hidden in
    the exp1->exp2 slack) so exp2 directly emits masked E bf16 and the
    exp2->ctx chain has no trim step; blocks with sk > sq are skipped,
  - heads 2-7 are "fused": q proj fully first, then k half 0, then scores
    strip-pairs each followed by one exp2 activation over both strips, so
    each head's E is ready during its own front and the ScalarE exp stream
    never lumps; softmax denominator comes free as a ones-column in v,
  - ctx accumulates on PSUM 2 bj per bank; ctx pairs of head h-2/h-1 and of
    head 7 itself are interleaved into the slots of the last fronts so only
    ctx7 p3 trails the last scores MM; the last pair borrows the idle psA
    pool (avoids psC rotation waits) and drains per-bj with the out DMA
    split across two queues; keep-warm dummy matmuls fill tail dependency
    stalls so the HAM clock gate stays at 2.4GHz,
  - output is written bf16 head-major partition-major; host reassembles+casts.
"""

import numpy as np
import ml_dtypes

import concourse.bass as bass
import concourse.bacc as bacc
import concourse.mybir as mybir
import concourse.tile as tile
from concourse.bass_utils import run_bass_kernel_spmd

B, S, FEAT, HID, H, DH = 8, 1024, 1024, 1024, 8, 128
NF = FEAT // 128  # f-blocks
NS = S // 128  # s-blocks
EOFF = [0]
for _bi in range(1, 9):
    EOFF.append(EOFF[-1] + 1024 - 128 * (_bi - 1))  # EOFF[8] = 4608
ELO = EOFF[4]  # 3328 cols, strips 0-3
EHI = EOFF[NS] - EOFF[4]  # 1280 cols, strips 4-7
SCALE = 1.0 / float(np.sqrt(DH))
F32 = mybir.dt.float32
BF16 = mybir.dt.bfloat16
BF_NP = ml_dtypes.bfloat16
EXP = mybir.ActivationFunctionType.Exp

_CACHED_NC = None


def build_nc():
    nc = bacc.Bacc()
    # all inputs packed partition-major on host for fat DMA descriptors
    # xt split into column halves so head 0 can start on partial data
    xt_d = nc.declare_dram_parameter("xt", [128, 2, NF, 512], BF16, isOutput=False)
    wqk_d = nc.declare_dram_parameter("wqk", [H, 128, 2, NF, 128], BF16, isOutput=False)
    wv_d = nc.declare_dram_parameter("wv", [128, NF, HID], BF16, isOutput=False)
    # constf columns: 0:8 bq^T, 8:16 bk^T, 16:144 trim
    constf_d = nc.declare_dram_parameter("constf", [128, 144], F32, isOutput=False)
    # bv broadcast, bf16 (halves the early DMA bytes)
    bvb_d = nc.declare_dram_parameter("bvb", [128, HID], BF16, isOutput=False)
    out_d = nc.declare_dram_parameter("out", [H, 128, NS, 128], BF16, isOutput=True)

    with tile.TileContext(nc) as tc:
        with (
            tc.tile_pool(name="static", bufs=1) as staticp,
            tc.tile_pool(name="ebuf", bufs=3) as ep,
            tc.tile_pool(name="outt", bufs=2) as outp,
            tc.tile_pool(name="rcp", bufs=4) as rcpp,
            tc.tile_pool(name="psA", bufs=2, space=bass.MemorySpace.PSUM) as psA,
            tc.tile_pool(name="psS", bufs=2, space=bass.MemorySpace.PSUM) as psS,
            tc.tile_pool(name="psC", bufs=2, space=bass.MemorySpace.PSUM) as psC,
        ):
            # all persistent bufs=1 pools share one pool (distinct tags);
            # fewer pools -> fewer framework semaphores to init/clear
            constp = xtp = wvp = wcolp = qkp = vp = stp_pool = staticp
            # PE warmup: dummy matmuls with no DMA deps bridge the DMA ramp and
            # flip the HAM clock gate to 2.4GHz before real matmuls arrive.
            # wu memsets go on gpsimd (its queue preamble finishes first).
            wu_a = constp.tile([128, 128], BF16, name="wu_a", tag="wu_a")
            nc.gpsimd.memset(wu_a[:], 0.0)
            wu_b = constp.tile([128, 256], BF16, name="wu_b", tag="wu_b")
            nc.gpsimd.memset(wu_b[:], 0.0)

            # ---- DMA schedule ----
            # Queues hold ~2 outstanding descriptors; aggregate HBM rate grows
            # with bytes in flight. Two-phase granularity: tiny first pieces
            # (fast start) then big bulk pieces (bandwidth).
            xt0s = {
                fb: xtp.tile([128, 512], BF16, name=f"xt0s{fb}", tag=f"xt0s{fb}")
                for fb in (0, 1, 4, 5)
            }
            xt0p = [
                xtp.tile([128, 2, 512], BF16, name=f"xt0p{g}", tag=f"xt0p{g}")
                for g in range(2)
            ]
            wc_of = {}
            wqk0 = wcolp.tile([128, 2, NF, 128], BF16, name="wqk", tag="wqk")
            wc_of[0] = wqk0
            constf = constp.tile([128, 144], F32, name="constf", tag="constf")
            bvb = constp.tile([128, HID], BF16, name="bvb", tag="bvb")
            wv_sb = wvp.tile([128, NF, HID], BF16, name="wv", tag="wv")

            # Early DMA carries ONLY what head 0/1 need: the DMA engines
            # round-robin across queues with no priority, so any later-needed
            # descriptor issued early steals HBM bandwidth from the critical
            # pieces. wv/bvb/wqk2+ are emitted from the scalar queue at later
            # schedule points (the ACT stream naturally delays their issue).
            xtq1 = [
                xtp.tile([128, 4, 512], BF16, name=f"xt1h{g}", tag=f"xt1h{g}")
                for g in range(2)
            ]
            # sync: c0 fb0, fb1 fine; c0 (fb2,3) pair; c1 fb0-3 quad
            nc.sync.dma_start(xt0s[0][:], xt_d[:, 0, 0, :])
            nc.sync.dma_start(xt0s[1][:], xt_d[:, 0, 1, :])
            nc.sync.dma_start(xt0p[0][:], xt_d[:, 0, 2:4, :])
            nc.sync.dma_start(xtq1[0][:], xt_d[:, 1, 0:4, :])
            # gpsimd: c0 fb4, fb5 fine; c0 (fb6,7) pair; c1 fb4-7 quad
            nc.gpsimd.dma_start(xt0s[4][:], xt_d[:, 0, 4, :])
            nc.gpsimd.dma_start(xt0s[5][:], xt_d[:, 0, 5, :])
            nc.gpsimd.dma_start(xt0p[1][:], xt_d[:, 0, 6:8, :])
            nc.gpsimd.dma_start(xtq1[1][:], xt_d[:, 1, 4:8, :])
            # scalar: wqk0 pieces ordered by demand (q fb0-3, k fb0-3 while
            # q consumes c0's early pieces, then the fb4-7 halves); constants
            nc.scalar.dma_start(wqk0[:, 0, 0:4, :], wqk_d[0][:, 0, 0:4, :])
            # constf (biases) right behind the first weight piece: the first
            # bias-add gates PE via the psA rotation two proj groups later
            nc.scalar.dma_start(constf[:], constf_d[:])
            nc.scalar.dma_start(wqk0[:, 1, 0:4, :], wqk_d[0][:, 1, 0:4, :])
            nc.scalar.dma_start(wqk0[:, 0, 4:8, :], wqk_d[0][:, 0, 4:8, :])
            nc.scalar.dma_start(wqk0[:, 1, 4:8, :], wqk_d[0][:, 1, 4:8, :])
            for h in range(1, H):
                wc_of[h] = wcolp.tile(
                    [128, 2, NF, 128], BF16, name="wqk", tag=f"wqk{h}"
                )
            nc.scalar.dma_start(wc_of[1][:], wqk_d[1])
            nc.scalar.dma_start(bvb[:], bvb_d[:])

            def warm(n):
                # keep-warm dummy MMs: fill PE idle in dependency stalls so
                # the HAM clock gate never re-throttles in the tail
                for _ in range(n):
                    wups = psA.tile([128, 512], F32, name="psA", tag="psA")
                    nc.tensor.matmul(
                        wups[:, 0:256], wu_a[:], wu_b[:], start=True, stop=True
                    )

            # warmup MMs emitted after the dma_starts so the PE-queue DMA
            # descriptors issue first; the MMs themselves have no DMA deps.
            for _ in range(20):
                wups = psA.tile([128, 512], F32, name="psA", tag="psA")
                nc.tensor.matmul(wups[:, 0:256], wu_a[:], wu_b[:], start=True, stop=True)

            bqt = constf[:, 0:8]
            bkt = constf[:, 8:16]
            trim = constf[:, 16:144]

            def xts(c, fb):
                if c == 0:
                    if fb in xt0s:
                        return xt0s[fb]
                    g = 0 if fb < 4 else 1
                    return xt0p[g][:, fb % 2, :]
                return xtq1[fb // 4][:, fb % 4, :]

            FB_ORDER = list(range(NF))

            # big persistent tensors
            qT = qkp.tile([128, H, S], BF16, name="qT", tag="qT")  # [dh, head, s]
            kT = qkp.tile([128, H, S], BF16, name="kT", tag="kT")
            vv = vp.tile([128, NS, H * 129], BF16, name="vv", tag="vv")

            # ones columns of v~ (denominator trick)
            ones_ap = vv.rearrange("p si (h d) -> p si h d", d=129)[:, :, :, 128:129]
            nc.vector.memset(ones_ap, 1.0)

            def proj_one(h, qk, c):
                # q or k projection for head h, S-half c: 8 MMs + 1 bias-add
                wqk = wc_of[h]
                bt, dest = (bqt, qT) if qk == 0 else (bkt, kT)
                ps = psA.tile([128, 512], F32, name="psA", tag="psA")
                for i, fb in enumerate(FB_ORDER):
                    nc.tensor.matmul(
                        ps[:],
                        wqk[:, qk, fb, :],
                        xts(c, fb)[:],
                        start=(i == 0),
                        stop=(i == NF - 1),
                    )
                nc.vector.tensor_scalar_add(
                    dest[:, h, c * 512 : (c + 1) * 512], ps[:], bt[:, h : h + 1]
                )

            def proj_v_group(si, cv):
                # one (si, cv) group of the v projection: 8 MMs + 1 bias-add
                ps = psA.tile([128, 512], F32, name="psA", tag="psA")
                ch, off = si // 4, (si % 4) * 128
                for fb in range(NF):
                    nc.tensor.matmul(
                        ps[:],
                        xts(ch, fb)[:, off : off + 128],
                        wv_sb[:, fb, cv * 512 : (cv + 1) * 512],
                        start=(fb == 0),
                        stop=(fb == NF - 1),
                    )
                dst = vv[:, si, :].rearrange("p (h d) -> p h d", d=129)[
                    :, 4 * cv : 4 * cv + 4, 0:128
                ]
                nc.vector.tensor_add(
                    dst,
                    ps[:].rearrange("p (h d) -> p h d", d=128),
                    bvb[:, cv * 512 : (cv + 1) * 512].rearrange(
                        "p (h d) -> p h d", d=128
                    ),
                )

            def eslice(E, bi, a, b):
                # packed-trapezoid slice of strip bi, local cols [a, b)
                t = E[0] if bi < 4 else E[1]
                o = EOFF[bi] - (0 if bi < 4 else ELO)
                return t[:, o + a : o + b]

            st_of = {}

            def scores_strips(h, b0, b1):
                # strips b0..b1-1: MMs into a 2-bank psum tile, ONE exp per
                # strip, then the causal -128 mask added to the diagonal block
                # of st (so exp2 directly emits masked E — no trim after exp2,
                # and the DVE add hides in the exp1->exp2 slack)
                stq = st_of[h]
                for bi in range(b0, b1):
                    lo = bi * 128
                    w = S - lo
                    ps = psS.tile([128, 1024], F32, name="psS", tag="psS")
                    for c0 in range(lo, S, 512):
                        n = min(512, S - c0)
                        nc.tensor.matmul(
                            ps[:, c0 - lo : c0 - lo + n],
                            kT[:, h, lo : lo + 128],
                            qT[:, h, c0 : c0 + n],
                            start=True,
                            stop=True,
                        )
                    nc.scalar.activation(
                        stq[:, EOFF[bi] : EOFF[bi] + w], ps[:, 0:w], EXP, scale=SCALE
                    )
                    dg = stq[:, EOFF[bi] : EOFF[bi] + 128]
                    nc.vector.tensor_add(dg, dg, trim[:])

            E_of = {}

            def exp2(h):
                # second exp: 2 batched activations st->E, then diagonal trims
                stq = st_of.pop(h)
                elo = ep.tile([128, ELO], BF16, name="Elo", tag="Elo")
                ehi = ep.tile([128, EHI], BF16, name="Ehi", tag="Ehi")
                E_of[h] = (elo, ehi)
                nc.scalar.activation(elo[:], stq[:, 0:ELO], EXP)
                nc.scalar.activation(ehi[:], stq[:, ELO:], EXP)

            oh_of = {}

            def ctx_pair(h, p, split_out=False, pool=None):
                # bj pair (2p, 2p+1): MMs accumulate 2 bj per psum bank,
                # one reciprocal for both, 2 normalizes
                E = E_of[h]
                if p == 0:
                    oh_of[h] = outp.tile([128, NS, 128], BF16, name="oh", tag="oh")
                oh = oh_of[h]
                # late pairs can use the (then-idle) psA pool so they don't
                # wait on psC's bufs=2 rotation through earlier pairs' drains
                if pool is None:
                    ps = psC.tile([128, 2, 129], F32, name="psC", tag="psC")
                else:
                    ps = pool.tile([128, 2, 129], F32, name="psC", tag="psA")

                def mms(j):
                    bj = 2 * p + j
                    for bi in range(bj + 1):
                        nc.tensor.matmul(
                            ps[:, j, :],
                            eslice(E, bi, (bj - bi) * 128, (bj - bi) * 128 + 128),
                            vv[:, bi, h * 129 : h * 129 + 129],
                            start=(bi == 0),
                            stop=(bi == bj),
                        )

                if split_out:
                    # last pair of the last head: drain + DMA per bj on two
                    # queues so the final transfer is a single parallel 64KB
                    rc = rcpp.tile([128, 2, 1], F32, name="rc", tag="rc")
                    for j in range(2):
                        bj = 2 * p + j
                        mms(j)
                        if j == 0:
                            warm(2)  # fill PE idle while bj7's E dependency lands
                        nc.vector.reciprocal(rc[:, j, :], ps[:, j, 128:129])
                        nc.vector.tensor_scalar_mul(
                            oh[:, bj, :], ps[:, j, 0:128], rc[:, j, :]
                        )
                        q = nc.gpsimd if j == 0 else nc.sync
                        q.dma_start(
                            out_d[h][:, bj : bj + 1, :], oh[:, bj : bj + 1, :]
                        )
                else:
                    mms(0)
                    mms(1)
                    rc = rcpp.tile([128, 2, 1], F32, name="rc", tag="rc")
                    nc.vector.reciprocal(rc[:], ps[:, :, 128:129])
                    for j in range(2):
                        bj = 2 * p + j
                        nc.vector.tensor_scalar_mul(
                            oh[:, bj, :], ps[:, j, 0:128], rc[:, j, :]
                        )
                    nc.sync.dma_start(
                        out_d[h][:, 2 * p : 2 * p + 2, :], oh[:, 2 * p : 2 * p + 2, :]
                    )
                if p == 3:
                    del oh_of[h]
                    del E_of[h]

            def exp2_span(h, b0, b1):
                # fused head: one exp2 ACT covering strips [b0, b1) (contiguous
                # in the packed E tile), then the diagonal trims on the
                # otherwise-idle gpsimd so the ACT->trim->ctx chain is short
                stq = st_of[h]
                lo_t = EOFF[b0] if b0 < 4 else EOFF[b0] - ELO
                hi_t = lo_t + (EOFF[b1] - EOFF[b0])
                t = E_of[h][0] if b0 < 4 else E_of[h][1]
                nc.scalar.activation(
                    t[:, lo_t:hi_t], stq[:, EOFF[b0] : EOFF[b1]], EXP
                )

            def exp2_pair(h, bp):
                exp2_span(h, 2 * bp, 2 * bp + 2)

            def head_front(h, extras, fuse=False):
                # emits head h's projections+scores with `extras` (ctx pairs /
                # proj_v groups of earlier work) interleaved between MM groups
                # so PSUM drains never stall the tensor FIFO.
                st_of[h] = stp_pool.tile([128, EOFF[NS]], F32, name="stq", tag="stq")
                it = iter(extras)

                def nxt():
                    f = next(it, None)
                    if f is not None:
                        f()

                if fuse:
                    # q fully first, then k half 0 -> strips 0-3 can start
                    # (scores need all of qT but only kT's own strip), so the
                    # exp chain starts ~2 proj groups earlier.
                    elo = ep.tile([128, ELO], BF16, name="Elo", tag="Elo")
                    ehi = ep.tile([128, EHI], BF16, name="Ehi", tag="Ehi")
                    E_of[h] = (elo, ehi)
                    proj_one(h, 0, 0)
                    nxt()
                    proj_one(h, 0, 1)
                    nxt()
                    proj_one(h, 1, 0)
                    nxt()
                    scores_strips(h, 0, 2)
                    exp2_pair(h, 0)
                    nxt()
                    nxt()
                    proj_one(h, 1, 1)
                    nxt()
                    for bp in range(1, 4):
                        if h == 7 and bp == 3:
                            # last pair of the last head: per-strip exp2 so
                            # the final ctx bj's dependencies resolve sooner
                            scores_strips(h, 6, 7)
                            exp2_span(h, 6, 7)
                            scores_strips(h, 7, 8)
                            exp2_span(h, 7, 8)
                        else:
                            scores_strips(h, 2 * bp, 2 * bp + 2)
                            exp2_pair(h, bp)
                        nxt()
                        nxt()
                    st_of.pop(h, None)
                else:
                    for c in range(2):
                        for qk in range(2):
                            proj_one(h, qk, c)
                            nxt()
                    scores_strips(h, 0, 4)
                    nxt()
                    scores_strips(h, 4, 8)
                    nxt()
                    exp2(h)
                for f in it:
                    f()

            # ---- emission schedule ----
            # Heads 2..7 are fused (per-strip-pair exp2) so each head's E is
            # ready during its own front; ctx lags 2 heads. Later-needed
            # weights issue from the scalar queue at staggered points.
            noop = lambda: None
            head_front(0, [])
            nc.scalar.dma_start(wc_of[2][:], wqk_d[2])
            nc.scalar.dma_start(wv_sb[:, :, 0:512], wv_d[:, :, 0:512])
            head_front(1, [])
            nc.scalar.dma_start(wc_of[3][:], wqk_d[3])
            nc.scalar.dma_start(wv_sb[:, :, 512:1024], wv_d[:, :, 512:1024])
            def pv_ctx_extras(c, g):
                # proj_v groups for half c interleaved with ctx pairs of head g;
                # ctx pair p needs vv strips <= 2p+1, i.e. pv groups 0..2p+1
                ex = []
                for si in range(NS):
                    ex.append(lambda si=si: proj_v_group(si, c))
                    if si % 2 == 1:
                        ex.append(lambda p=si // 2: ctx_pair(g, p))
                return ex

            head_front(2, pv_ctx_extras(0, 0), fuse=True)
            nc.scalar.dma_start(wc_of[4][:], wqk_d[4])
            head_front(3, pv_ctx_extras(1, 1), fuse=True)
            nc.scalar.dma_start(wc_of[5][:], wqk_d[5])
            head_front(
                4,
                [lambda p=p: ctx_pair(2, p) for p in range(4)] + [noop] * 8,
                fuse=True,
            )
            nc.scalar.dma_start(wc_of[6][:], wqk_d[6])
            head_front(
                5,
                [lambda p=p: ctx_pair(3, p) for p in range(4)] + [noop] * 8,
                fuse=True,
            )
            nc.scalar.dma_start(wc_of[7][:], wqk_d[7])
            head_front(
                6,
                [lambda p=p: ctx_pair(4, p) for p in range(4)]
                + [lambda p=p: ctx_pair(5, p) for p in range(4)]
                + [noop] * 4,
                fuse=True,
            )
            # head 7: ctx6 pairs fill the proj slots (E6 is ready early since
            # head 6 was fused); head 7's own ctx pairs run one strip-pair
            # behind their E strips so only ctx7 p3 trails the last scores MM.
            head_front(
                7,
                [
                    lambda: ctx_pair(6, 0),
                    lambda: ctx_pair(6, 1),
                    lambda: ctx_pair(6, 2),
                    lambda: ctx_pair(6, 3),
                    # strip-pair slots, 2 per pair; warm() only where stalls
                    # actually occur (late pairs), so ready work isn't delayed
                    noop,
                    noop,  # bp=0
                    lambda: ctx_pair(7, 0),
                    lambda: warm(4),  # bp=1
                    lambda: ctx_pair(7, 1),
                    lambda: warm(4),  # bp=2
                    lambda: ctx_pair(7, 2),
                    lambda: warm(4),  # bp=3
                ],
                fuse=True,
            )
            warm(2)
            ctx_pair(7, 3, split_out=True, pool=psA)

    nc.finalize()
    return nc


def _get_nc():
    global _CACHED_NC
    if _CACHED_NC is None:
        _CACHED_NC = build_nc()
    return _CACHED_NC


def _prep_shared(Wq, bq, Wk, bk, Wv, bv):
    def reorder(w):
        # [f, n] -> [h, f_in_blk(partition), f_blk, c]
        return w.reshape(NF, 128, H, 128).transpose(2, 1, 0, 3)

    wqk = np.ascontiguousarray(
        np.stack([reorder(Wq), reorder(Wk)], axis=2)
    ).astype(BF_NP)  # [H, 128, 2, NF, 128]
    wvh = np.ascontiguousarray(
        Wv.reshape(NF, 128, HID).transpose(1, 0, 2)
    ).astype(BF_NP)  # [128, NF, HID]
    constf = np.empty((128, 144), np.float32)
    constf[:, 0:8] = bq.reshape(H, 128).T
    constf[:, 8:16] = bk.reshape(H, 128).T
    # additive causal mask for the diagonal st block: -128 where sk > sq
    # (partition = sk, free col = sq), matching the reference's soft mask
    constf[:, 16:144] = np.tril(
        np.full((128, 128), -128.0, dtype=np.float32), k=-1
    )
    bvb = np.ascontiguousarray(np.broadcast_to(bv, (128, HID))).astype(BF_NP)
    return dict(wqk=wqk, wv=wvh, constf=constf, bvb=bvb)


def _prep_xt(x):
    # [S, F] -> xT [F, S] -> [128(p), c, NF, 512] partition-major, bf16
    return np.ascontiguousarray(
        x.T.reshape(NF, 128, 2, 512).transpose(1, 2, 0, 3)
    ).astype(BF_NP)


def _unpack_out(arr):
    # [H, 128, NS, 128] -> [S, HID]
    return np.ascontiguousarray(
        arr.transpose(2, 1, 0, 3).reshape(S, HID).astype(np.float32)
    )


def kernel(queries, Wq, bq, Wk, bk, Wv, bv):
    queries = np.asarray(queries, np.float32)
    shared = _prep_shared(
        np.asarray(Wq, np.float32),
        np.asarray(bq, np.float32),
        np.asarray(Wk, np.float32),
        np.asarray(bk, np.float32),
        np.asarray(Wv, np.float32),
        np.asarray(bv, np.float32),
    )
    in_maps = [dict(xt=_prep_xt(queries[b]), **shared) for b in range(B)]
    nc = _get_nc()
    res = run_bass_kernel_spmd(nc, in_maps, core_ids=list(range(B)))
    return np.stack([_unpack_out(res.results[b]["out"]) for b in range(B)], axis=0)


if __name__ == "__main__":
    rng = np.random.default_rng(0)
    q = rng.standard_normal((B, S, FEAT), dtype=np.float32)
    Wq = (rng.standard_normal((FEAT, HID), dtype=np.float32) * 0.02).astype(np.float32)
    Wk = (rng.standard_normal((FEAT, HID), dtype=np.float32) * 0.02).astype(np.float32)
    Wv = (rng.standard_normal((FEAT, HID), dtype=np.float32) * 0.02).astype(np.float32)
    z = np.zeros(HID, np.float32)
    out = kernel(queries=q, Wq=Wq, bq=z, Wk=Wk, bk=z, Wv=Wv, bv=z)
    print(out.shape, out.dtype)

